# revision 41
# baseline (speedup 1.0000x reference)
"""Chunked attention kernel for Trainium2 (Bass/Tile), SPMD over 8 NeuronCores.

Problem (hardcoded):
  x: [B=8, C=1024, L=4096] fp32, Wq/Wk/Wv/Wo: [1024,1024] fp32 (stored [in,out]),
  biases [1024] fp32.  H=8 heads, head_dim=128, CHUNK=64 (block-diagonal attention).
  out = transpose(softmax((xt@Wq)(xt@Wk)^T/sqrt(128) blockwise) @ (xt@Wv) @ Wo, [B,C,L])

Sharding: data-parallel over B — one batch per core. No collectives.

The four C=1024-contraction projections run as fp8-e4m3 DoubleRow matmuls with a
3-term split-precision expansion:
    x@W ~= xh@Wh + xh@Wl + xl@Wh
where xh = fp8(x), xl = fp8(x - xh) (host-side), Wh = fp8(32*W),
Wl = fp8(32*W - Wh). DoubleRow contracts two 128-k-planes per instruction at
0.5 cycles/column, so the 3-term sum costs 0.75x the fp16 cycles while keeping
the dropped xl@Wl term ~0.13%. The 32x weight scale (needed to keep W out of
e4m3 subnormals) cancels: scores pick up 1/1024 inside the exp scale, the
output projection result is scaled by 1/1024 at eviction.

Engine split: PE does matmuls; Pool (gpsimd queue) evicts Q/K/V/O PSUM tiles
(0.83ns/col, no PSUM-access bubble, and it drains in parallel with DVE); DVE
keeps softmax reciprocal + EN mul + the fp8 split of P; ACT does the exps.
Scores for head h-1 are emitted between Q-proj(h) and K-proj(h) so the Pool
eviction latency is covered by PE work instead of stalling the score matmuls.

Per-core dataflow (PSUM fp32 accumulate):
  Q~^T[c,l] = 32*Q^T via DR matmuls (lhsT=Wq pair-tiles, rhs=x pair views)
  K~^T, V~[l,c] likewise (V token-major: lhsT=x, rhs=Wv)
  per head h, chunk-pair p (128 tokens):
    S~^T[k,q] = matmul(lhsT=K~^T block, rhs=Q~^T block)  fp16  (= 1024*S^T)
    E = exp(S~^T * scale/1024) on the two diagonal 64x64 blocks, rest zero
    D = matmul(lhsT=ones, rhs=E) -> denominator replicated on all partitions
    R = 1/D, EN = E*R  (normalized attn, transposed)
    P~^T[d,q] = matmul(lhsT=V~ block, rhs=EN) = 32*P^T  -> split to fp8 ph+pl
  out^T[c,l] = DR-matmuls(lhsT=Wo pairs, rhs=ph/pl pairs) * (1/1024)
"""

import numpy as np
import ml_dtypes
from contextlib import ExitStack

import concourse.bass as bass
import concourse.bacc as bacc
import concourse.tile as tile
import concourse.mybir as mybir

B, C, L = 8, 1024, 4096
H, HD, CHUNK, PAIR = 8, 128, 64, 128
N_CORES = 8
KT = C // 128          # 8 contraction planes
NDR = KT // 2          # 4 DoubleRow plane-pairs
LT = 512               # tokens per strip
F8 = mybir.dt.float8e4
F16 = mybir.dt.float16
F32 = mybir.dt.float32
NP8 = ml_dtypes.float8_e4m3
DR = mybir.MatmulPerfMode.DoubleRow
SCALE = 1.0 / float(np.sqrt(HD))
WNAMES = ("wq", "wk", "wv", "wo")
WSCALE = 32.0          # host-side weight scale before fp8 quantization
OUT_DESCALE = 1.0 / (WSCALE * WSCALE)
# 3-term split order: (x-level, w-level); hh first, then lh (xl arrives
# before wq_l at startup), then hl
TERMS = (("h", "h"), ("l", "h"), ("h", "l"))


def _emit(ctx, tc, x_d, w_d, o_d, l_total):
    nc = tc.nc
    NS = l_total // LT     # strips
    NP = LT // PAIR        # chunk-pairs (= token 128-tiles) per strip

    wpool = ctx.enter_context(tc.tile_pool(name="w", bufs=1))
    cpool = ctx.enter_context(tc.tile_pool(name="const", bufs=1))
    xpool = ctx.enter_context(tc.tile_pool(name="xp", bufs=2))
    qpool = ctx.enter_context(tc.tile_pool(name="qp", bufs=2))
    vpool = ctx.enter_context(tc.tile_pool(name="vp", bufs=2))
    epool = ctx.enter_context(tc.tile_pool(name="ep", bufs=1))
    rpool = ctx.enter_context(tc.tile_pool(name="rp", bufs=1))
    npool = ctx.enter_context(tc.tile_pool(name="np", bufs=1))
    ppool = ctx.enter_context(tc.tile_pool(name="pp", bufs=2))
    opool = ctx.enter_context(tc.tile_pool(name="op", bufs=1))
    pjps = ctx.enter_context(tc.tile_pool(name="pj", bufs=4, space="PSUM"))
    scps = ctx.enter_context(tc.tile_pool(name="sc", bufs=2, space="PSUM"))
    pvps = ctx.enter_context(tc.tile_pool(name="pv", bufs=2, space="PSUM"))

    # --- weights: one [128, KT*C] fp8 tile per (name, level); DR pair t lives
    # --- at cols [t*2C, (t+1)*2C) with plane 2t then 2t+1
    wt = {}

    def load_w(n, lvl, halves=1):
        tl = wpool.tile([128, KT * C], F8, tag=f"{n}{lvl}")
        hw = KT // (2 * halves)   # DR pairs per DMA
        for b in range(halves):
            nc.sync.dma_start(
                tl[:, b * hw * 2 * C:(b + 1) * hw * 2 * C].rearrange(
                    "p (t i c) -> p t i c", i=2, c=C),
                w_d[(n, lvl)].rearrange("(t i p) c -> p t i c", p=128, i=2)[
                    :, b * hw:(b + 1) * hw])
        wt[(n, lvl)] = tl

    def wv_(n, lvl, t, cols):
        # lhsT/rhs pair view [128, 2, ncols] of weight pair t
        return wt[(n, lvl)][:, t * 2 * C:(t + 1) * 2 * C].rearrange(
            "p (two c) -> p two c", two=2)[:, :, cols]

    def xv_(x_t, t, cols=slice(0, LT)):
        # pair view [128, 2, ncols] of an x-layout tile (planes at j*LT)
        return x_t[:, 2 * t * LT:(2 * t + 2) * LT].rearrange(
            "p (two n) -> p two n", two=2)[:, :, cols]

    def load_x(s):
        xh_t = xpool.tile([128, KT * LT], F8, tag="xh")
        xl_t = xpool.tile([128, KT * LT], F8, tag="xl")
        for lvl, t in (("h", xh_t), ("l", xl_t)):
            nc.sync.dma_start(
                t[:].rearrange("p (j c) -> p j c", c=LT),
                x_d[lvl].rearrange("(j p) l -> p j l", p=128)[:, :, s * LT:(s + 1) * LT])
        return xh_t, xl_t

    def proj_group(ps, nm, xs, t_cols, w_cols, x_is_lhs=False, terms=TERMS):
        n = 0
        for xl_, wl_ in terms:
            for t in range(NDR):
                a = wv_(nm, wl_, t, w_cols)
                b = xv_(xs[xl_], t, t_cols)
                lhsT, rhs = (b, a) if x_is_lhs else (a, b)
                nc.tensor.matmul(ps, lhsT, rhs,
                                 start=(n == 0),
                                 stop=(n == len(terms) * NDR - 1),
                                 perf_mode=DR)
                n += 1

    def o_proj_m(o_t, ph, pl, ls, m, dma_every=4):
        xs = {"h": ph, "l": pl}
        ps = pjps.tile([128, 512], F32, tag="pj")
        proj_group(ps[:, 0:LT], "wo", xs, slice(0, LT),
                   slice(m * 128, (m + 1) * 128))
        nc.scalar.activation(o_t[:, m * LT:(m + 1) * LT], ps[:, 0:LT],
                             mybir.ActivationFunctionType.Copy,
                             scale=OUT_DESCALE)
        if m % dma_every == dma_every - 1:
            m0 = m - dma_every + 1
            nc.sync.dma_start(
                o_d.rearrange("(m p) l -> p m l", p=128)[
                    :, m0:m + 1, ls:ls + LT],
                o_t[:, m0 * LT:(m + 1) * LT].rearrange(
                    "p (m c) -> p m c", c=LT))

    # Startup DMAs in exact first-use order, split in halves so the first
    # matmuls of each projection group start sooner: the h=0 Q group needs
    # xh, wq_h, xl, wq_l; K follows; V/O land during the head loop.
    xh_t0 = xpool.tile([128, KT * LT], F8, tag="xh")
    xl_t0 = xpool.tile([128, KT * LT], F8, tag="xl")

    def load_x0(t, lvl, b):
        nc.sync.dma_start(
            t[:, b * 4 * LT:(b + 1) * 4 * LT].rearrange("p (j c) -> p j c", c=LT),
            x_d[lvl].rearrange("(j p) l -> p j l", p=128)[:, b * 4:(b + 1) * 4, 0:LT])

    load_x0(xh_t0, "h", 0)
    load_x0(xh_t0, "h", 1)
    load_w("wq", "h", halves=2)
    load_x0(xl_t0, "l", 0)
    load_x0(xl_t0, "l", 1)
    load_w("wq", "l", halves=2)
    load_w("wk", "h")
    load_w("wk", "l")
    load_w("wv", "h")
    load_w("wv", "l")
    load_w("wo", "h")
    load_w("wo", "l")
    x_next = (xh_t0, xl_t0)
    ones_t = cpool.tile([128, PAIR], F16, tag="ones")
    nc.gpsimd.memset(ones_t[:], 1.0)
    ones = ones_t[:]
    # [ones | zeros] fp8 pair: DoubleRow lhsT for the denominator colsums (the
    # zero slot voids the second plane's contribution, halving the PE cost)
    oz_t = cpool.tile([128, 2 * PAIR], F8, tag="oz")
    nc.gpsimd.memset(oz_t[:, 0:PAIR], 1.0)
    nc.gpsimd.memset(oz_t[:, PAIR:2 * PAIR], 0.0)
    bias_t = cpool.tile([128, 1], F32, tag="bias")
    nc.gpsimd.memset(bias_t[:], -float(np.log(32.0)))
    # e_t is a single persistent buffer: exps rewrite the diagonal blocks every
    # strip, the off-diagonal stays zero from this one memset. e8_t mirrors it
    # in fp8 (padded one junk group) to feed the DoubleRow colsum.
    e_t = epool.tile([128, H * LT], F16, tag="e")
    nc.gpsimd.memset(e_t[:], 0.0)
    e8_t = epool.tile([128, (H + 1) * LT], F8, tag="e8")
    nc.gpsimd.memset(e8_t[:], 0.0)
    # warm up the PE p-state while the startup DMAs land: one long PSUM
    # accumulation group (no per-matmul group semaphores = no micro-gaps, so
    # the clock ramp is continuous and the first projections run full speed)
    warm = scps.tile([128, LT], F32, tag="sc")
    NWARM = 48
    for i in range(NWARM):
        nc.tensor.matmul(warm[:, 0:PAIR], ones, ones,
                         start=(i == 0), stop=(i == NWARM - 1))

    for s in range(NS):
        ls = s * LT
        xh_t, xl_t = x_next
        xs = {"h": xh_t, "l": xl_t}

        qk_t = qpool.tile([128, 2 * KT * LT], F16, tag="qk")
        r_t = rpool.tile([128, H * LT], F16, tag="r")
        en_t = npool.tile([128, H * LT], F16, tag="en")

        def denom_group(g, pool=None):
            # pvps is idle during the head/V phases; keeping denominators out
            # of pjps relieves the projection-group psum rotation. Trailing
            # denominators pass pjps instead so they don't hold both pvps
            # buffers right when PV(0) needs one.
            ps = (pool or pvps).tile([128, NP * PAIR], F32,
                                     tag="pv" if pool is None else "pj")
            # colsum via fp8 DoubleRow: slot0 = ones x E8 group, slot1 zeroed
            nc.tensor.matmul(
                ps[:],
                oz_t[:].rearrange("p (two c) -> p two c", two=2),
                e8_t[:, g * 512:(g + 2) * 512].rearrange(
                    "p (two c) -> p two c", two=2),
                start=True, stop=True, perf_mode=DR)
            with nc.allow_low_precision(reason="softmax recip fp16 ample"):
                nc.vector.reciprocal(r_t[:, g * 512:(g + 1) * 512], ps[:])
            # EN = E * (1/D): all-SBUF fp16, runs on the otherwise idle Pool
            nc.gpsimd.tensor_mul(en_t[:, g * 512:(g + 1) * 512],
                                 e_t[:, g * 512:(g + 1) * 512],
                                 r_t[:, g * 512:(g + 1) * 512])

        def scores_exp(h):
            sc = scps.tile([128, LT], F32, tag="sc")
            qb = h * 2 * LT
            kb = qb + LT
            for p in range(NP):
                nc.tensor.matmul(sc[:, p * PAIR:(p + 1) * PAIR],
                                 qk_t[:, kb + p * PAIR:kb + (p + 1) * PAIR],
                                 qk_t[:, qb + p * PAIR:qb + (p + 1) * PAIR],
                                 start=True, stop=True)
            # exp of the diagonal 64x64 blocks of every pair -> e_t (off-diag
            # stays 0). One strided ACT per half: [64, (pairs), 64] pattern.
            # bias -ln(32) shifts E into [~0, ~10]: softmax is shift-invariant
            # and the fp8 shadow copy stays far from e4m3's 240 max
            eh = e_t[:, h * LT:(h + 1) * LT]
            for r0, c0 in ((0, 0), (64, 64)):
                nc.scalar.activation(
                    eh[r0:r0 + 64, :].rearrange("a (np c) -> a np c", c=PAIR)[:, :, c0:c0 + 64],
                    sc[r0:r0 + 64, :].rearrange("a (np c) -> a np c", c=PAIR)[:, :, c0:c0 + 64],
                    mybir.ActivationFunctionType.Exp,
                    scale=SCALE / (WSCALE * WSCALE),
                    bias=bias_t[r0:r0 + 64, :])
            # fp8 shadow of E for the DoubleRow denominator (Pool, off the
            # critical path; 3.6%/sqrt(64) ~ 0.7% error on D is well in budget)
            with nc.allow_low_precision(reason="fp8 denominator input"):
                nc.gpsimd.tensor_copy(e8_t[:, h * LT:(h + 1) * LT], eh)

        # --- Q/K projections, with head h-1's scores emitted between Q(h) and
        # --- K(h): the eviction of K(h-1) completes under Q(h)'s matmuls.
        # --- Q evicts ride ACT, K evicts ride DVE: parallel queues.
        # --- Strip 0 runs all Q groups first: the wk DMAs are still in
        # --- flight, and 10us of Q matmuls cover them.
        def q_part(h):
            ps = pjps.tile([128, 512], F32, tag="pj")
            proj_group(ps[:, 0:LT], "wq", xs, slice(0, LT),
                       slice(h * 128, (h + 1) * 128))
            nc.scalar.activation(qk_t[:, h * 2 * LT:h * 2 * LT + LT],
                                 ps[:, 0:LT],
                                 mybir.ActivationFunctionType.Copy, scale=1.0)

        def k_part(h):
            ps = pjps.tile([128, 512], F32, tag="pj")
            proj_group(ps[:, 0:LT], "wk", xs, slice(0, LT),
                       slice(h * 128, (h + 1) * 128))
            nc.vector.tensor_copy(qk_t[:, h * 2 * LT + LT:(h + 1) * 2 * LT],
                                  ps[:, 0:LT])

        if s == 0:
            # Two-phase strip 0: the hh-only partial groups need just xh+wq_h
            # (the first DMAs to land); the lh/hl remainder accumulates into
            # qk_t via a DVE add once xl/wq_l arrive. No mid-group DMA stalls,
            # so the PE clock ramp never resets.
            for nm, off_k, ev in (("wq", 0, "act"), ("wk", LT, "dve")):
                for h in range(H):
                    ps = pjps.tile([128, 512], F32, tag="pj")
                    proj_group(ps[:, 0:LT], nm, xs, slice(0, LT),
                               slice(h * 128, (h + 1) * 128), terms=TERMS[:1])
                    dst = qk_t[:, h * 2 * LT + off_k:h * 2 * LT + off_k + LT]
                    if ev == "act":
                        nc.scalar.activation(
                            dst, ps[:, 0:LT],
                            mybir.ActivationFunctionType.Copy, scale=1.0)
                    else:
                        nc.vector.tensor_copy(dst, ps[:, 0:LT])
            for nm, off_k in (("wq", 0), ("wk", LT)):
                for h in range(H):
                    ps = pjps.tile([128, 512], F32, tag="pj")
                    proj_group(ps[:, 0:LT], nm, xs, slice(0, LT),
                               slice(h * 128, (h + 1) * 128), terms=TERMS[1:])
                    dst = qk_t[:, h * 2 * LT + off_k:h * 2 * LT + off_k + LT]
                    nc.vector.tensor_add(dst, ps[:, 0:LT], dst)
                    if nm == "wk":
                        if h >= 1:
                            scores_exp(h - 1)
                        if h >= 3:
                            denom_group(h - 3)
        else:
            for h in range(H):
                q_part(h)
                if h >= 1:
                    scores_exp(h - 1)
                k_part(h)
                # denominators: group g == head g at LT=512; exp(h-3) landed
                # two iterations ago, so the colsum never waits on the ACT
                if h >= 3:
                    denom_group(h - 3)

        # --- V projection (token-major): V~[l, c] per 128-token tile
        v_t = vpool.tile([128, NP * C], F16, tag="v")
        for p in range(NP):
            for n2 in range(C // 512):
                ps = pjps.tile([128, 512], F32, tag="pj")
                proj_group(ps[:], "wv", xs,
                           slice(p * 128, (p + 1) * 128),
                           slice(n2 * 512, (n2 + 1) * 512), x_is_lhs=True)
                nc.scalar.activation(
                    v_t[:, p * C + n2 * 512:p * C + (n2 + 1) * 512], ps[:],
                    mybir.ActivationFunctionType.Copy, scale=1.0)
                if p == 0 and n2 == 0:
                    scores_exp(H - 1)
            if p == 0:
                denom_group(H - 3)
        # trailing denominators right before PV: their PE colsums cover the
        # ACT V-eviction latency at the V->PV boundary
        for g in range(H - 2, H):
            denom_group(g, pool=pjps)

        # prefetch next strip's x now that V-proj consumed this strip's
        if s + 1 < NS:
            x_next = load_x(s + 1)

        # --- attention output P~^T[d, q] interleaved with the PREVIOUS strip's
        # --- output projection (software pipelining): per head, 4 PV matmuls
        # --- + one O m-group on PE while DVE does the fp8 split
        # --- (ph = fp8(ps), pl = fp8(ps - ph)) and ACT the O eviction
        ph_t = ppool.tile([128, KT * LT], F8, tag="ph")
        pl_t = ppool.tile([128, KT * LT], F8, tag="pl")
        if s >= 1:
            o_t = opool.tile([128, KT * 512], F16, tag="o")
        for h in range(H):
            ps = pvps.tile([128, NP * PAIR], F32, tag="pv")
            for p in range(NP):
                nc.tensor.matmul(ps[:, p * PAIR:(p + 1) * PAIR],
                                 v_t[:, p * C + h * 128:p * C + (h + 1) * 128],
                                 en_t[:, h * LT + p * PAIR:h * LT + (p + 1) * PAIR],
                                 start=True, stop=True)
            if s >= 1:
                o_proj_m(o_t, ph_prev, pl_prev, ls_prev, h)
            with nc.allow_low_precision(reason="fp8 split-precision eviction"):
                nc.scalar.activation(ph_t[:, h * LT:(h + 1) * LT], ps[:],
                                     mybir.ActivationFunctionType.Copy,
                                     scale=1.0)
                nc.vector.tensor_sub(pl_t[:, h * LT:(h + 1) * LT], ps[:],
                                     ph_t[:, h * LT:(h + 1) * LT])
        ph_prev, pl_prev, ls_prev = ph_t, pl_t, ls
    o_t = opool.tile([128, KT * 512], F16, tag="o")
    for m in range(KT - 1):
        o_proj_m(o_t, ph_prev, pl_prev, ls_prev, m, dma_every=1)
    # final block in two half-column groups: the first half's eviction + DMA
    # overlap the second half's matmuls, shortening the end-of-program drain
    m = KT - 1
    xs_f = {"h": ph_prev, "l": pl_prev}
    hf = LT // 2
    for b in range(2):
        ps = pjps.tile([128, 512], F32, tag="pj")
        proj_group(ps[:, 0:hf], "wo", xs_f, slice(b * hf, (b + 1) * hf),
                   slice(m * 128, (m + 1) * 128))
        nc.scalar.activation(o_t[:, m * LT + b * hf:m * LT + (b + 1) * hf],
                             ps[:, 0:hf], mybir.ActivationFunctionType.Copy,
                             scale=OUT_DESCALE)
        nc.sync.dma_start(
            o_d[m * 128:(m + 1) * 128,
                ls_prev + b * hf:ls_prev + (b + 1) * hf],
            o_t[:, m * LT + b * hf:m * LT + (b + 1) * hf])


def build_nc(l_total=L):
    nc = bacc.Bacc("TRN2", target_bir_lowering=False, debug=False,
                   enable_asserts=False)
    x_d = {lvl: nc.dram_tensor(f"x{lvl}", [C, l_total], F8, kind="ExternalInput").ap()
           for lvl in ("h", "l")}
    w_d = {(n, lvl): nc.dram_tensor(f"{n}{lvl}", [C, C], F8, kind="ExternalInput").ap()
           for n in WNAMES for lvl in ("h", "l")}
    o_d = nc.dram_tensor("out", [C, l_total], F16, kind="ExternalOutput").ap()
    with tile.TileContext(nc) as tc:
        with ExitStack() as ctx:
            _emit(ctx, tc, x_d, w_d, o_d, l_total)
    nc.compile()
    return nc


_NC_CACHE = {}


def _get_nc(l_total):
    if l_total not in _NC_CACHE:
        _NC_CACHE[l_total] = build_nc(l_total)
    return _NC_CACHE[l_total]


def make_in_maps(x, Wq, Wk, Wv, Wo):
    x = np.asarray(x, np.float32)
    xh = x.astype(NP8)
    xl = (x - xh.astype(np.float32)).astype(NP8)
    ws = {}
    for n, w in zip(WNAMES, (Wq, Wk, Wv, Wo)):
        g = np.asarray(w, np.float32) * WSCALE
        wh = g.astype(NP8)
        wl = (g - wh.astype(np.float32)).astype(NP8)
        ws[f"{n}h"] = np.ascontiguousarray(wh)
        ws[f"{n}l"] = np.ascontiguousarray(wl)
    in_maps = []
    for i in range(x.shape[0]):
        m = {"xh": np.ascontiguousarray(xh[i]), "xl": np.ascontiguousarray(xl[i])}
        m.update(ws)
        in_maps.append(m)
    return in_maps


def _numpy_fallback(x, Wq, bq, Wk, bk, Wv, bv, Wo, bo):
    # Exact host-side path, used only if biases are nonzero (the problem spec
    # fills them with zeros, so the device kernel does not apply them).
    x = np.asarray(x, np.float32)
    Bn, Cn, Ln = x.shape
    hd = Cn // H
    nch = Ln // CHUNK
    xt = np.transpose(x, (0, 2, 1))
    Q = (xt @ Wq + bq).reshape(Bn, nch, CHUNK, H, hd)
    K = (xt @ Wk + bk).reshape(Bn, nch, CHUNK, H, hd)
    V = (xt @ Wv + bv).reshape(Bn, nch, CHUNK, H, hd)
    scores = np.einsum("bnqhd,bnkhd->bnhqk", Q, K) / np.sqrt(hd)
    scores -= scores.max(axis=-1, keepdims=True)
    e = np.exp(scores)
    attn = e / e.sum(axis=-1, keepdims=True)
    out = np.einsum("bnhqk,bnkhd->bnqhd", attn, V).reshape(Bn, Ln, Cn)
    out = out @ Wo + bo
    return np.ascontiguousarray(np.transpose(out, (0, 2, 1)).astype(np.float32))


def kernel(x, Wq, bq, Wk, bk, Wv, bv, Wo, bo, trace=False):
    from concourse.bass_utils import run_bass_kernel_spmd
    nb, c_in, l_total = x.shape
    if (any(np.any(np.asarray(b) != 0) for b in (bq, bk, bv, bo))
            or c_in != C or l_total % LT != 0 or nb > N_CORES):
        return _numpy_fallback(x, Wq, bq, Wk, bk, Wv, bv, Wo, bo)
    nc = _get_nc(l_total)
    in_maps = make_in_maps(x, Wq, Wk, Wv, Wo)
    res = run_bass_kernel_spmd(nc, in_maps, core_ids=list(range(nb)), trace=trace)
    out = np.stack([res.results[i]["out"] for i in range(nb)], axis=0).astype(np.float32)
    if trace:
        return out, res
    return out


# revision 42
# speedup vs baseline: 1.0256x; 1.0256x over previous
"""Chunked attention kernel for Trainium2 (Bass/Tile), SPMD over 8 NeuronCores.

Problem (hardcoded):
  x: [B=8, C=1024, L=4096] fp32, Wq/Wk/Wv/Wo: [1024,1024] fp32 (stored [in,out]),
  biases [1024] fp32.  H=8 heads, head_dim=128, CHUNK=64 (block-diagonal attention).
  out = transpose(softmax((xt@Wq)(xt@Wk)^T/sqrt(128) blockwise) @ (xt@Wv) @ Wo, [B,C,L])

Sharding: data-parallel over B — one batch per core. No collectives.

The four C=1024-contraction projections run as fp8-e4m3 DoubleRow matmuls with a
3-term split-precision expansion:
    x@W ~= xh@Wh + xh@Wl + xl@Wh
where xh = fp8(x), xl = fp8(x - xh) (host-side), Wh = fp8(32*W),
Wl = fp8(32*W - Wh). DoubleRow contracts two 128-k-planes per instruction at
0.5 cycles/column, so the 3-term sum costs 0.75x the fp16 cycles while keeping
the dropped xl@Wl term ~0.13%. The 32x weight scale (needed to keep W out of
e4m3 subnormals) cancels: scores pick up 1/1024 inside the exp scale, the
output projection result is scaled by 1/1024 at eviction.

Engine split: PE does matmuls; Pool (gpsimd queue) evicts Q/K/V/O PSUM tiles
(0.83ns/col, no PSUM-access bubble, and it drains in parallel with DVE); DVE
keeps softmax reciprocal + EN mul + the fp8 split of P; ACT does the exps.
Scores for head h-1 are emitted between Q-proj(h) and K-proj(h) so the Pool
eviction latency is covered by PE work instead of stalling the score matmuls.

Per-core dataflow (PSUM fp32 accumulate):
  Q~^T[c,l] = 32*Q^T via DR matmuls (lhsT=Wq pair-tiles, rhs=x pair views)
  K~^T, V~[l,c] likewise (V token-major: lhsT=x, rhs=Wv)
  per head h, chunk-pair p (128 tokens):
    S~^T[k,q] = matmul(lhsT=K~^T block, rhs=Q~^T block)  fp16  (= 1024*S^T)
    E = exp(S~^T * scale/1024) on the two diagonal 64x64 blocks, rest zero
    D = matmul(lhsT=ones, rhs=E) -> denominator replicated on all partitions
    R = 1/D, EN = E*R  (normalized attn, transposed)
    P~^T[d,q] = matmul(lhsT=V~ block, rhs=EN) = 32*P^T  -> split to fp8 ph+pl
  out^T[c,l] = DR-matmuls(lhsT=Wo pairs, rhs=ph/pl pairs) * (1/1024)
"""

import numpy as np
import ml_dtypes
from contextlib import ExitStack

import concourse.bass as bass
import concourse.bacc as bacc
import concourse.tile as tile
import concourse.mybir as mybir

B, C, L = 8, 1024, 4096
H, HD, CHUNK, PAIR = 8, 128, 64, 128
N_CORES = 8
KT = C // 128          # 8 contraction planes
NDR = KT // 2          # 4 DoubleRow plane-pairs
LT = 512               # tokens per strip
F8 = mybir.dt.float8e4
F16 = mybir.dt.float16
F32 = mybir.dt.float32
NP8 = ml_dtypes.float8_e4m3
DR = mybir.MatmulPerfMode.DoubleRow
SCALE = 1.0 / float(np.sqrt(HD))
WNAMES = ("wq", "wk", "wv", "wo")
WSCALE = 32.0          # host-side weight scale before fp8 quantization
OUT_DESCALE = 1.0 / (WSCALE * WSCALE)
# 3-term split order: (x-level, w-level); hh first, then lh (xl arrives
# before wq_l at startup), then hl
TERMS = (("h", "h"), ("l", "h"), ("h", "l"))


def _emit(ctx, tc, x_d, w_d, o_d, l_total):
    nc = tc.nc
    NS = l_total // LT     # strips
    NP = LT // PAIR        # chunk-pairs (= token 128-tiles) per strip

    wpool = ctx.enter_context(tc.tile_pool(name="w", bufs=1))
    cpool = ctx.enter_context(tc.tile_pool(name="const", bufs=1))
    xpool = ctx.enter_context(tc.tile_pool(name="xp", bufs=2))
    qpool = ctx.enter_context(tc.tile_pool(name="qp", bufs=2))
    vpool = ctx.enter_context(tc.tile_pool(name="vp", bufs=2))
    epool = ctx.enter_context(tc.tile_pool(name="ep", bufs=1))
    rpool = ctx.enter_context(tc.tile_pool(name="rp", bufs=1))
    npool = ctx.enter_context(tc.tile_pool(name="np", bufs=1))
    ppool = ctx.enter_context(tc.tile_pool(name="pp", bufs=2))
    opool = ctx.enter_context(tc.tile_pool(name="op", bufs=1))
    pjps = ctx.enter_context(tc.tile_pool(name="pj", bufs=4, space="PSUM"))
    scps = ctx.enter_context(tc.tile_pool(name="sc", bufs=2, space="PSUM"))
    pvps = ctx.enter_context(tc.tile_pool(name="pv", bufs=2, space="PSUM"))

    # --- weights: one [128, KT*C] fp8 tile per (name, level); DR pair t lives
    # --- at cols [t*2C, (t+1)*2C) with plane 2t then 2t+1
    wt = {}

    def load_w(n, lvl, halves=1):
        tl = wpool.tile([128, KT * C], F8, tag=f"{n}{lvl}")
        hw = KT // (2 * halves)   # DR pairs per DMA
        for b in range(halves):
            nc.sync.dma_start(
                tl[:, b * hw * 2 * C:(b + 1) * hw * 2 * C].rearrange(
                    "p (t i c) -> p t i c", i=2, c=C),
                w_d[(n, lvl)].rearrange("(t i p) c -> p t i c", p=128, i=2)[
                    :, b * hw:(b + 1) * hw])
        wt[(n, lvl)] = tl

    def wv_(n, lvl, t, cols):
        # lhsT/rhs pair view [128, 2, ncols] of weight pair t
        return wt[(n, lvl)][:, t * 2 * C:(t + 1) * 2 * C].rearrange(
            "p (two c) -> p two c", two=2)[:, :, cols]

    def xv_(x_t, t, cols=slice(0, LT)):
        # pair view [128, 2, ncols] of an x-layout tile (planes at j*LT)
        return x_t[:, 2 * t * LT:(2 * t + 2) * LT].rearrange(
            "p (two n) -> p two n", two=2)[:, :, cols]

    def load_x(s):
        xh_t = xpool.tile([128, KT * LT], F8, tag="xh")
        xl_t = xpool.tile([128, KT * LT], F8, tag="xl")
        for lvl, t in (("h", xh_t), ("l", xl_t)):
            nc.sync.dma_start(
                t[:].rearrange("p (j c) -> p j c", c=LT),
                x_d[lvl].rearrange("(j p) l -> p j l", p=128)[:, :, s * LT:(s + 1) * LT])
        return xh_t, xl_t

    def proj_group(ps, nm, xs, t_cols, w_cols, x_is_lhs=False):
        n = 0
        for xl_, wl_ in TERMS:
            for t in range(NDR):
                a = wv_(nm, wl_, t, w_cols)
                b = xv_(xs[xl_], t, t_cols)
                lhsT, rhs = (b, a) if x_is_lhs else (a, b)
                nc.tensor.matmul(ps, lhsT, rhs,
                                 start=(n == 0),
                                 stop=(n == 3 * NDR - 1),
                                 perf_mode=DR)
                n += 1

    def o_proj_m(o_t, ph, pl, ls, m, dma_every=4):
        xs = {"h": ph, "l": pl}
        ps = pjps.tile([128, 512], F32, tag="pj")
        proj_group(ps[:, 0:LT], "wo", xs, slice(0, LT),
                   slice(m * 128, (m + 1) * 128))
        nc.scalar.activation(o_t[:, m * LT:(m + 1) * LT], ps[:, 0:LT],
                             mybir.ActivationFunctionType.Copy,
                             scale=OUT_DESCALE)
        if m % dma_every == dma_every - 1:
            m0 = m - dma_every + 1
            nc.sync.dma_start(
                o_d.rearrange("(m p) l -> p m l", p=128)[
                    :, m0:m + 1, ls:ls + LT],
                o_t[:, m0 * LT:(m + 1) * LT].rearrange(
                    "p (m c) -> p m c", c=LT))

    # Startup DMAs in exact first-use order, split in halves so the first
    # matmuls of each projection group start sooner: the h=0 Q group needs
    # xh, wq_h, xl, wq_l; K follows; V/O land during the head loop.
    xh_t0 = xpool.tile([128, KT * LT], F8, tag="xh")
    xl_t0 = xpool.tile([128, KT * LT], F8, tag="xl")

    def load_x0(t, lvl, b):
        nc.sync.dma_start(
            t[:, b * 4 * LT:(b + 1) * 4 * LT].rearrange("p (j c) -> p j c", c=LT),
            x_d[lvl].rearrange("(j p) l -> p j l", p=128)[:, b * 4:(b + 1) * 4, 0:LT])

    load_x0(xh_t0, "h", 0)
    load_x0(xh_t0, "h", 1)
    load_w("wq", "h", halves=2)
    load_x0(xl_t0, "l", 0)
    load_x0(xl_t0, "l", 1)
    load_w("wq", "l", halves=2)
    load_w("wk", "h")
    load_w("wk", "l")
    load_w("wv", "h")
    load_w("wv", "l")
    load_w("wo", "h")
    load_w("wo", "l")
    x_next = (xh_t0, xl_t0)
    ones_t = cpool.tile([128, PAIR], F16, tag="ones")
    nc.gpsimd.memset(ones_t[:], 1.0)
    ones = ones_t[:]
    # [ones | zeros] fp8 pair: DoubleRow lhsT for the denominator colsums (the
    # zero slot voids the second plane's contribution, halving the PE cost)
    oz_t = cpool.tile([128, 2 * PAIR], F8, tag="oz")
    nc.gpsimd.memset(oz_t[:, 0:PAIR], 1.0)
    nc.gpsimd.memset(oz_t[:, PAIR:2 * PAIR], 0.0)
    bias_t = cpool.tile([128, 1], F32, tag="bias")
    nc.gpsimd.memset(bias_t[:], -float(np.log(32.0)))
    # e_t is a single persistent buffer: exps rewrite the diagonal blocks every
    # strip, the off-diagonal stays zero from this one memset. e8_t mirrors it
    # in fp8 (padded one junk group) to feed the DoubleRow colsum.
    e_t = epool.tile([128, H * LT], F16, tag="e")
    nc.gpsimd.memset(e_t[:], 0.0)
    e8_t = epool.tile([128, (H + 1) * LT], F8, tag="e8")
    nc.gpsimd.memset(e8_t[:], 0.0)
    # warm up the PE p-state while the startup DMAs land: one long PSUM
    # accumulation group (no per-matmul group semaphores = no micro-gaps, so
    # the clock ramp is continuous and the first projections run full speed)
    warm = scps.tile([128, LT], F32, tag="sc")
    NWARM = 48
    for i in range(NWARM):
        nc.tensor.matmul(warm[:, 0:PAIR], ones, ones,
                         start=(i == 0), stop=(i == NWARM - 1))

    for s in range(NS):
        ls = s * LT
        xh_t, xl_t = x_next
        xs = {"h": xh_t, "l": xl_t}

        qk_t = qpool.tile([128, 2 * KT * LT], F16, tag="qk")
        r_t = rpool.tile([128, H * LT], F16, tag="r")
        en_t = npool.tile([128, H * LT], F16, tag="en")

        def denom_group(g, pool=None):
            # pvps is idle during the head/V phases; keeping denominators out
            # of pjps relieves the projection-group psum rotation. Trailing
            # denominators pass pjps instead so they don't hold both pvps
            # buffers right when PV(0) needs one.
            ps = (pool or pvps).tile([128, NP * PAIR], F32,
                                     tag="pv" if pool is None else "pj")
            # colsum via fp8 DoubleRow: slot0 = ones x E8 group, slot1 zeroed
            nc.tensor.matmul(
                ps[:],
                oz_t[:].rearrange("p (two c) -> p two c", two=2),
                e8_t[:, g * 512:(g + 2) * 512].rearrange(
                    "p (two c) -> p two c", two=2),
                start=True, stop=True, perf_mode=DR)
            with nc.allow_low_precision(reason="softmax recip fp16 ample"):
                nc.vector.reciprocal(r_t[:, g * 512:(g + 1) * 512], ps[:])
            # EN = E * (1/D): all-SBUF fp16, runs on the otherwise idle Pool
            nc.gpsimd.tensor_mul(en_t[:, g * 512:(g + 1) * 512],
                                 e_t[:, g * 512:(g + 1) * 512],
                                 r_t[:, g * 512:(g + 1) * 512])

        def scores_exp(h):
            sc = scps.tile([128, LT], F32, tag="sc")
            qb = h * 2 * LT
            kb = qb + LT
            for p in range(NP):
                nc.tensor.matmul(sc[:, p * PAIR:(p + 1) * PAIR],
                                 qk_t[:, kb + p * PAIR:kb + (p + 1) * PAIR],
                                 qk_t[:, qb + p * PAIR:qb + (p + 1) * PAIR],
                                 start=True, stop=True)
            # exp of the diagonal 64x64 blocks of every pair -> e_t (off-diag
            # stays 0). One strided ACT per half: [64, (pairs), 64] pattern.
            # bias -ln(32) shifts E into [~0, ~10]: softmax is shift-invariant
            # and the fp8 shadow copy stays far from e4m3's 240 max
            eh = e_t[:, h * LT:(h + 1) * LT]
            for r0, c0 in ((0, 0), (64, 64)):
                nc.scalar.activation(
                    eh[r0:r0 + 64, :].rearrange("a (np c) -> a np c", c=PAIR)[:, :, c0:c0 + 64],
                    sc[r0:r0 + 64, :].rearrange("a (np c) -> a np c", c=PAIR)[:, :, c0:c0 + 64],
                    mybir.ActivationFunctionType.Exp,
                    scale=SCALE / (WSCALE * WSCALE),
                    bias=bias_t[r0:r0 + 64, :])
            # fp8 shadow of E for the DoubleRow denominator (Pool, off the
            # critical path; 3.6%/sqrt(64) ~ 0.7% error on D is well in budget)
            with nc.allow_low_precision(reason="fp8 denominator input"):
                nc.gpsimd.tensor_copy(e8_t[:, h * LT:(h + 1) * LT], eh)

        # --- Q/K projections, with head h-1's scores emitted between Q(h) and
        # --- K(h): the eviction of K(h-1) completes under Q(h)'s matmuls.
        # --- Q evicts ride ACT, K evicts ride DVE: parallel queues.
        # --- Strip 0 runs all Q groups first: the wk DMAs are still in
        # --- flight, and 10us of Q matmuls cover them.
        def q_part(h):
            ps = pjps.tile([128, 512], F32, tag="pj")
            proj_group(ps[:, 0:LT], "wq", xs, slice(0, LT),
                       slice(h * 128, (h + 1) * 128))
            nc.scalar.activation(qk_t[:, h * 2 * LT:h * 2 * LT + LT],
                                 ps[:, 0:LT],
                                 mybir.ActivationFunctionType.Copy, scale=1.0)

        def k_part(h):
            ps = pjps.tile([128, 512], F32, tag="pj")
            proj_group(ps[:, 0:LT], "wk", xs, slice(0, LT),
                       slice(h * 128, (h + 1) * 128))
            nc.vector.tensor_copy(qk_t[:, h * 2 * LT + LT:(h + 1) * 2 * LT],
                                  ps[:, 0:LT])

        if s == 0:
            for h in range(H):
                q_part(h)
            for h in range(H):
                k_part(h)
                if h >= 1:
                    scores_exp(h - 1)
                if h >= 3:
                    denom_group(h - 3)
        else:
            for h in range(H):
                q_part(h)
                if h >= 1:
                    scores_exp(h - 1)
                k_part(h)
                # denominators: group g == head g at LT=512; exp(h-3) landed
                # two iterations ago, so the colsum never waits on the ACT
                if h >= 3:
                    denom_group(h - 3)

        # --- V projection (token-major): V~[l, c] per 128-token tile
        v_t = vpool.tile([128, NP * C], F16, tag="v")
        for p in range(NP):
            for n2 in range(C // 512):
                ps = pjps.tile([128, 512], F32, tag="pj")
                proj_group(ps[:], "wv", xs,
                           slice(p * 128, (p + 1) * 128),
                           slice(n2 * 512, (n2 + 1) * 512), x_is_lhs=True)
                nc.scalar.activation(
                    v_t[:, p * C + n2 * 512:p * C + (n2 + 1) * 512], ps[:],
                    mybir.ActivationFunctionType.Copy, scale=1.0)
                if p == 0 and n2 == 0:
                    scores_exp(H - 1)
            if p == 0:
                denom_group(H - 3)
        # trailing denominators right before PV: their PE colsums cover the
        # ACT V-eviction latency at the V->PV boundary
        for g in range(H - 2, H):
            denom_group(g, pool=pjps)

        # prefetch next strip's x now that V-proj consumed this strip's
        if s + 1 < NS:
            x_next = load_x(s + 1)

        # --- attention output P~^T[d, q] interleaved with the PREVIOUS strip's
        # --- output projection (software pipelining): per head, 4 PV matmuls
        # --- + one O m-group on PE while DVE does the fp8 split
        # --- (ph = fp8(ps), pl = fp8(ps - ph)) and ACT the O eviction
        ph_t = ppool.tile([128, KT * LT], F8, tag="ph")
        pl_t = ppool.tile([128, KT * LT], F8, tag="pl")
        if s >= 1:
            o_t = opool.tile([128, KT * 512], F16, tag="o")
        for h in range(H):
            ps = pvps.tile([128, NP * PAIR], F32, tag="pv")
            for p in range(NP):
                nc.tensor.matmul(ps[:, p * PAIR:(p + 1) * PAIR],
                                 v_t[:, p * C + h * 128:p * C + (h + 1) * 128],
                                 en_t[:, h * LT + p * PAIR:h * LT + (p + 1) * PAIR],
                                 start=True, stop=True)
            if s >= 1:
                o_proj_m(o_t, ph_prev, pl_prev, ls_prev, h)
            with nc.allow_low_precision(reason="fp8 split-precision eviction"):
                nc.scalar.activation(ph_t[:, h * LT:(h + 1) * LT], ps[:],
                                     mybir.ActivationFunctionType.Copy,
                                     scale=1.0)
                nc.vector.tensor_sub(pl_t[:, h * LT:(h + 1) * LT], ps[:],
                                     ph_t[:, h * LT:(h + 1) * LT])
        ph_prev, pl_prev, ls_prev = ph_t, pl_t, ls
    o_t = opool.tile([128, KT * 512], F16, tag="o")
    for m in range(KT - 1):
        o_proj_m(o_t, ph_prev, pl_prev, ls_prev, m, dma_every=1)
    # final block in two half-column groups: the first half's eviction + DMA
    # overlap the second half's matmuls, shortening the end-of-program drain
    m = KT - 1
    xs_f = {"h": ph_prev, "l": pl_prev}
    hf = LT // 2
    for b in range(2):
        ps = pjps.tile([128, 512], F32, tag="pj")
        proj_group(ps[:, 0:hf], "wo", xs_f, slice(b * hf, (b + 1) * hf),
                   slice(m * 128, (m + 1) * 128))
        nc.scalar.activation(o_t[:, m * LT + b * hf:m * LT + (b + 1) * hf],
                             ps[:, 0:hf], mybir.ActivationFunctionType.Copy,
                             scale=OUT_DESCALE)
        nc.sync.dma_start(
            o_d[m * 128:(m + 1) * 128,
                ls_prev + b * hf:ls_prev + (b + 1) * hf],
            o_t[:, m * LT + b * hf:m * LT + (b + 1) * hf])


def build_nc(l_total=L):
    nc = bacc.Bacc("TRN2", target_bir_lowering=False, debug=False,
                   enable_asserts=False)
    x_d = {lvl: nc.dram_tensor(f"x{lvl}", [C, l_total], F8, kind="ExternalInput").ap()
           for lvl in ("h", "l")}
    w_d = {(n, lvl): nc.dram_tensor(f"{n}{lvl}", [C, C], F8, kind="ExternalInput").ap()
           for n in WNAMES for lvl in ("h", "l")}
    o_d = nc.dram_tensor("out", [C, l_total], F16, kind="ExternalOutput").ap()
    with tile.TileContext(nc) as tc:
        with ExitStack() as ctx:
            _emit(ctx, tc, x_d, w_d, o_d, l_total)
    nc.compile()
    return nc


_NC_CACHE = {}


def _get_nc(l_total):
    if l_total not in _NC_CACHE:
        _NC_CACHE[l_total] = build_nc(l_total)
    return _NC_CACHE[l_total]


def make_in_maps(x, Wq, Wk, Wv, Wo):
    x = np.asarray(x, np.float32)
    xh = x.astype(NP8)
    xl = (x - xh.astype(np.float32)).astype(NP8)
    ws = {}
    for n, w in zip(WNAMES, (Wq, Wk, Wv, Wo)):
        g = np.asarray(w, np.float32) * WSCALE
        wh = g.astype(NP8)
        wl = (g - wh.astype(np.float32)).astype(NP8)
        ws[f"{n}h"] = np.ascontiguousarray(wh)
        ws[f"{n}l"] = np.ascontiguousarray(wl)
    in_maps = []
    for i in range(x.shape[0]):
        m = {"xh": np.ascontiguousarray(xh[i]), "xl": np.ascontiguousarray(xl[i])}
        m.update(ws)
        in_maps.append(m)
    return in_maps


def _numpy_fallback(x, Wq, bq, Wk, bk, Wv, bv, Wo, bo):
    # Exact host-side path, used only if biases are nonzero (the problem spec
    # fills them with zeros, so the device kernel does not apply them).
    x = np.asarray(x, np.float32)
    Bn, Cn, Ln = x.shape
    hd = Cn // H
    nch = Ln // CHUNK
    xt = np.transpose(x, (0, 2, 1))
    Q = (xt @ Wq + bq).reshape(Bn, nch, CHUNK, H, hd)
    K = (xt @ Wk + bk).reshape(Bn, nch, CHUNK, H, hd)
    V = (xt @ Wv + bv).reshape(Bn, nch, CHUNK, H, hd)
    scores = np.einsum("bnqhd,bnkhd->bnhqk", Q, K) / np.sqrt(hd)
    scores -= scores.max(axis=-1, keepdims=True)
    e = np.exp(scores)
    attn = e / e.sum(axis=-1, keepdims=True)
    out = np.einsum("bnhqk,bnkhd->bnqhd", attn, V).reshape(Bn, Ln, Cn)
    out = out @ Wo + bo
    return np.ascontiguousarray(np.transpose(out, (0, 2, 1)).astype(np.float32))


def kernel(x, Wq, bq, Wk, bk, Wv, bv, Wo, bo, trace=False):
    from concourse.bass_utils import run_bass_kernel_spmd
    nb, c_in, l_total = x.shape
    if (any(np.any(np.asarray(b) != 0) for b in (bq, bk, bv, bo))
            or c_in != C or l_total % LT != 0 or nb > N_CORES):
        return _numpy_fallback(x, Wq, bq, Wk, bk, Wv, bv, Wo, bo)
    nc = _get_nc(l_total)
    in_maps = make_in_maps(x, Wq, Wk, Wv, Wo)
    res = run_bass_kernel_spmd(nc, in_maps, core_ids=list(range(nb)), trace=trace)
    out = np.stack([res.results[i]["out"] for i in range(nb)], axis=0).astype(np.float32)
    if trace:
        return out, res
    return out
